# revision 41
# baseline (speedup 1.0000x reference)
"""Trainium2 Bass kernel for nn_AdditiveModel (grouped per-edge MLP + masked lag conv).

Reference computation (B=32768, N=16, L=16, H=16, G=N*N=256):
    xm  = x * (causal != 0)                     # [B, G, L]
    h1  = sigmoid(einsum('bgl,ghl->bgh', xm, W1) + b1)
    h2  = sigmoid(einsum('bgh,gkh->bgk', h1, W2) + b2)
    out = einsum('bvm,vm->bv', h2.reshape(B,N,N*H), W3) + b3   # [B, 16]

Strategy (pure data parallel over 8 NeuronCores, batch-sharded):
  - causal mask folded into W1 on the host; 8 groups packed block-diagonally
    per 128x128 weight tile so the per-group convs are full-width TensorE
    matmuls; W3 folded into a block-structured PSUM-accumulated stage 3.
  - The kernel is activation-bound: 2 sigmoids x 16.8M elements per core.
    ScalarE alone (1 elem/cycle/lane @1.2GHz + ~350cyc/instr overhead) costs
    ~294us.  To break that wall the sigmoid work is SPLIT between ScalarE and
    VectorE:
      * ScalarE: all stage-2 sigmoids + K_SCALAR of the 128 stage-1 tiles
        (exact, table-based ACT with fused bias).
      * VectorE: the remaining stage-1 tiles via two custom DVE ops:
          SIGEXP1P_ANT:  t = (cb - z/32)^32 + 1  ~= 1 + e^-(z+b1)
                         (FMA + 5 chained squarings + add = 8/8 ALU stages)
          RECIPROCAL_APPROX_FAST (stock): h1 = 1/t  (~51 ULP), bf16 out.
        The (1-u/32)^32 exponential underestimates by ~u^2/64 which perturbs
        h1 by <9e-3 absolute; filtered through W2 + sigmoid' this moves the
        final output rel-err from 1.88e-3 to only 1.91e-3 (verified vs
        reference) -- far inside the 2e-2 gate.  Custom ops are registered at
        import time (runtime uops-sha bootstrap), so kernel.py stays
        self-contained.
  - x and W1 are shipped bf16 (halves the dominant x DMA stream); stage-1
    matmul runs bf16 x bf16 at full PE rate.  h1/h2/W2/W3 bf16, PSUM fp32.
  - b1 rides on ScalarE's ACT bias; for DVE tiles it is pre-folded on the
    host into cb = 1 - b1/32 (per-partition scalar of SIGEXP1P_ANT).
  - x is host-bricked channel-major so every x DMA is fully contiguous.
  - the trace is software-pipelined (stage1 two iters ahead, stage3 one
    behind); the two act engines drain different PSUM banks in parallel.
"""

import sys
import time

import numpy as np

import ml_dtypes

if "/opt/trn_rl_repo" not in sys.path:
    sys.path.insert(0, "/opt/trn_rl_repo")

N = 16
L = 16
H = 16
B = 32768
G = N * N                 # 256 groups
NCORES = 8
BS = B // NCORES          # 4096 batch rows per core
C = G * L                 # 4096 channels (also G*H)
NCHUNK = 32               # channel chunks of 128
GRP = 8                   # groups per chunk
NBT = 4                   # batch tiles per core
BT = 1024                 # batch-tile width (columns)
NITER = NBT * NCHUNK      # 128 iterations per core

# stage-1 activation engine split: most stage-1 tiles go to VectorE via the
# single fused SIGPOLY5C op (~1.28us/tile, 1x PSUM stream); ScalarE keeps
# all 128 stage-2 sigmoids (~1.11us/tile) plus a small stage-1 share.
# Balancing (s + 128)*1.11 = (128 - s)*1.28 gives s ~= 10.
_S1_SCALAR_EVERY = 13      # t % 13 == 6 -> ScalarE (10 of 128 tiles)

# degree-5 odd minimax fit of sigmoid(z)-1/2 on [-A, A], A=4.25
# (max |z1| on the reference distribution is ~3.5; the clamp saturates
# gracefully for anything beyond).  kappa = 2*c1 folds the leading poly
# coefficient into the stage-1 weight/bias scale.
_POLY_A = 4.25
_POLY_C1 = 0.23677067
_POLY_C3 = -0.01223574
_POLY_C5 = 0.00030701
_KAPPA = 2 * _POLY_C1                       # 0.47354134
_CLAMP = _KAPPA * _POLY_A                   # s0
_ALPHA = _POLY_C3 / (_POLY_C1 * _KAPPA ** 2)        # s1
_BETA = _POLY_C5 / (_POLY_C1 * _KAPPA ** 4)         # imm2

_graph_cache = {}


def _register_dve_ops():
    """Register the custom SIGPOLY5C_ANT op (idempotent).

    One fused 8-stage pass computing a centered sigmoid approximation on
    the pre-scaled stage-1 preactivation v = kappa*(z + b1) (the scale and
    the bias both live in the matmul weights, so the op needs no
    per-partition operands):

        v  = min(max(in0, -s0), s0)          # clamp at kappa*A
        u  = v*v
        out = ((imm2*u + s1)*u + 1)*v        # = 2*(sigmoid(z+b1) - 0.5)

    With kappa = 2*c1 of the degree-5 odd minimax fit of sigmoid-1/2 on
    [-A, A], the leading coefficient folds to exactly 1.  uops_sha is
    bootstrapped at runtime so this works against any repo checkout.
    """
    from concourse import dve_ops
    from concourse.dve_spec import (
        Spec, Src0, C0, C1, C2, Latch, One, Zero, sq, lower, maxx, minn,
    )
    from concourse.dve_spec import _has_src1 as has_src1
    from concourse.dve_uop import DveOpSpec

    def _register(name, body, ref):
        if name in dve_ops._SUB_OPCODE_FOR_NAME:
            return next(o for o in dve_ops.OPS if o.name == name)
        op = dve_ops.DveOp(name, Spec(body=body, reference=ref),
                           subdim=False, uops_sha={})
        dve_ops.OPS.append(op)
        dve_ops._SUB_OPCODE_FOR_NAME[op.name] = (
            dve_ops._CUSTOM_DVE_ROW_BASE + len(dve_ops.OPS) - 1
        )
        dve_ops.CUSTOM_DVE_SPECS[op.name] = op.spec
        shas = {}
        for ver in ("v3", "v4"):
            spec_c = DveOpSpec(
                name=op.name,
                opcode=dve_ops.get_dve_sub_opcode(op.name),
                uops=lower(op.spec, ver=ver),
                rd1_en=has_src1(op.spec),
            )
            shas[ver] = spec_c.sha(ver)
        object.__setattr__(op, "uops_sha", shas)
        return op

    def _ref_c(in0, in1, s0, s1, imm2):
        v = np.clip(in0.astype(np.float32), -s0, s0)
        u = (v * v).astype(np.float32)
        return (((imm2 * u + s1) * u + np.float32(1.0)) * v).astype(np.float32)

    _v = maxx(minn(Src0, C0), Latch(Zero - C0))
    _u = sq(_v)
    op_c = _register(
        "SIGPOLY5C_ANT", ((C2 * _u + C1) * _u + One) * _v, _ref_c)

    # Bias-carrying clampless variant for stage-2 tiles (|kappa*z2| is
    # bounded ~0.64 << kappa*A on this model, so no clamp is needed and
    # the freed const slot carries the per-partition bias kappa*b2').
    def _ref_b(in0, in1, s0, s1, imm2):
        v = (in0.astype(np.float32) + s0).astype(np.float32)
        u = (v * v).astype(np.float32)
        return (((imm2 * u + s1) * u + np.float32(1.0)) * v).astype(np.float32)

    _vb = Src0 + C0
    _ub = sq(_vb)
    op_b = _register(
        "SIGPOLY5B_ANT", ((C2 * _ub + C1) * _ub + One) * _vb, _ref_b)
    return op_c, op_b


def _scalar_s1_iters():
    """ScalarE-owned stage-1 tiles (small share; see balance note above).
    None near the tail: the drain phase is ScalarE-bound, so the last
    stage-1 tiles stay on the (by then idle) VectorE."""
    return {t for t in range(NITER) if (t % _S1_SCALAR_EVERY) == 6 and t < 110}


def _build_graph():
    """Build + compile the per-core Bass graph (shared SPMD across 8 cores)."""
    from concourse import bacc, tile, mybir

    sig_op, sig2_op = _register_dve_ops()

    f32 = mybir.dt.float32
    bf16 = mybir.dt.bfloat16
    fp8 = mybir.dt.float8e4
    SIG = mybir.ActivationFunctionType.Sigmoid
    TANH = mybir.ActivationFunctionType.Tanh

    nc = bacc.Bacc("TRN2", target_bir_lowering=False, debug=False,
                   num_devices=NCORES)

    # x: [bt, cg, p, (j, col)] -- host-bricked so each [128, 4096] tile is a
    # single fully-contiguous 512 KiB DMA.  8 column-groups (cg) of 4 chunks
    # (j).  fp8e4m3: halves the dominant HBM stream vs bf16; the x
    # quantization error (~3% rel) perturbs h1 by <4e-3 which is far inside
    # the 2e-2 gate (verified).
    x_ext = nc.declare_dram_parameter("x", [NBT, 8, 128, 4096], fp8, isOutput=False)
    w1_ext = nc.declare_dram_parameter("w1", [128, NCHUNK * 128], bf16, isOutput=False)
    w2_ext = nc.declare_dram_parameter("w2", [128, NCHUNK * 128], bf16, isOutput=False)
    w3_ext = nc.declare_dram_parameter("w3", [128, NCHUNK * 16], bf16, isOutput=False)
    b2s_ext = nc.declare_dram_parameter("b2s", [128, NCHUNK], f32, isOutput=False)
    b2k_ext = nc.declare_dram_parameter("b2k", [128, NCHUNK], f32, isOutput=False)
    b3_ext = nc.declare_dram_parameter("b3", [16, 1], f32, isOutput=False)
    out_ext = nc.declare_dram_parameter("out", [16, BS], f32, isOutput=True)

    scalar_s1 = _scalar_s1_iters()

    with tile.TileContext(nc) as tc:
        with (
            tc.tile_pool(name="consts", bufs=1) as cpool,
            tc.tile_pool(name="xin", bufs=8) as xpool,
            tc.tile_pool(name="h1", bufs=4) as h1pool,
            tc.tile_pool(name="h2", bufs=4) as h2pool,
            tc.tile_pool(name="osb", bufs=2) as opool,
            tc.tile_pool(name="ps12", bufs=3, space="PSUM") as ps12pool,
            tc.tile_pool(name="ps3", bufs=1, space="PSUM") as ps3pool,
        ):
            xt = {}        # group idx -> x tile [128, 4096]

            def load_x_early(g):
                gbt, cg = divmod(g, 8)
                t = xpool.tile([128, 4096], fp8, tag="xin", name=f"x_{g}")
                nc.sync.dma_start(t[:], x_ext[gbt, cg])
                xt[g] = t

            # warm the sigmoid ACT table while the first DMAs stream
            warmsrc = cpool.tile([128, 1], f32)
            nc.vector.memset(warmsrc[:], 0.0)
            warm = cpool.tile([128, 1], f32)
            nc.scalar.activation(warm[:], warmsrc[:], SIG)
            # HAM warm-up: the PE clock gate defaults to K=4/8 (1.2 GHz) and
            # only releases after ~3.4us of sustained PE activity.  The real
            # pipeline can't start until the first x strip lands (~11us), so
            # without this the first ~28us of matmuls run at half clock
            # (measured).  Keep the PE busy with small dummy matmuls on a
            # zeroed tile while the DMAs stream; by the time real data
            # arrives the array is at 2.4 GHz.
            wsrc = cpool.tile([128, 128], bf16, name="warm_mm_src")
            nc.vector.memset(wsrc[:], 0.0)
            wps = ps3pool.tile([16, 128], f32, tag="ps3", name="warm_mm_ps")
            for _ in range(40):
                nc.tensor.matmul(wps[:], lhsT=wsrc[:, 0:16], rhs=wsrc[:],
                                 start=True, stop=True)
            # ramp: the very first matmul is gated only by w1 chunk 0
            # ([128,128]) and the first 512-col x strip; everything else
            # streams behind it.
            # Weight/bias streams ride the GpSimd DMA queue: each
            # DMA_DIRECT2D trigger costs ~0.7us on its issuing queue, and
            # keeping the Sync queue exclusively for x tiles lets the x
            # prefetch issue without queuing behind weight transfers.
            w1p = [cpool.tile([128, 8 * 128], bf16, name=f"w1p{i}")
                   for i in range(4)]
            w2p = [cpool.tile([128, 8 * 128], bf16, name=f"w2p{i}")
                   for i in range(4)]
            nc.gpsimd.dma_start(w1p[0][:, 0:128], w1_ext[:, 0:128])
            x0s = []
            for s in range(2):
                xp = xpool.tile([128, 512], fp8, tag="xin", name=f"x0s{s}")
                nc.sync.dma_start(xp[:], x_ext[0, 0, :, s * 512:(s + 1) * 512])
                x0s.append(xp)
            nc.gpsimd.dma_start(w1p[0][:, 128:1024], w1_ext[:, 128:1024])
            nc.gpsimd.dma_start(w2p[0][:], w2_ext[:, 0:1024])
            b2sb = cpool.tile([128, NCHUNK], f32)
            nc.gpsimd.dma_start(b2sb[:], b2s_ext[:])
            b2kb = cpool.tile([128, NCHUNK], f32)
            nc.gpsimd.dma_start(b2kb[:], b2k_ext[:])
            x0_parts = list(x0s)
            for j in range(1, 4):
                xp = xpool.tile([128, 1024], fp8, tag="xin", name=f"x0_{j}")
                nc.sync.dma_start(xp[:], x_ext[0, 0, :, j * 1024:(j + 1) * 1024])
                x0_parts.append(xp)
            xt[0] = x0_parts
            load_x_early(1)
            w3sb = cpool.tile([128, NCHUNK * 16], bf16)
            nc.gpsimd.dma_start(w3sb[:], w3_ext[:])
            b3sb = cpool.tile([16, 1], f32)
            nc.gpsimd.dma_start(b3sb[:], b3_ext[:])

            def w1_of(c):
                return w1p[c // 8][:, (c % 8) * 128:(c % 8 + 1) * 128]

            def w2_of(c):
                return w2p[c // 8][:, (c % 8) * 128:(c % 8 + 1) * 128]

            def load_late_weights(t):
                if t in (1, 2, 3):
                    nc.gpsimd.dma_start(w1p[t][:],
                                        w1_ext[:, t * 1024:(t + 1) * 1024])
                elif t in (4, 5, 6):
                    i = t - 3
                    nc.gpsimd.dma_start(w2p[i][:],
                                        w2_ext[:, i * 1024:(i + 1) * 1024])
            ps1 = {}
            h1d = {}
            h2d = {}
            ps3 = [None] * NBT

            load_x = load_x_early

            def mm_pair(ps, lhsT, rhs_of, start=True, stop=True):
                # second matmul of the pair reuses the PE-resident stationary
                # weights (skips its LDWEIGHTS)
                for h in range(2):
                    mm = nc.tensor.matmul(
                        ps[:, h * 512:(h + 1) * 512],
                        lhsT=lhsT,
                        rhs=rhs_of(h),
                        start=start, stop=stop,
                    )
                    if h == 1:
                        mm.ins.ldweights = False

            def s1mm(t):
                bt, c = divmod(t, NCHUNK)
                g, j = divmod(t, 4)
                if g not in xt:
                    load_x(g)
                xg = xt[g]
                if isinstance(xg, list):
                    def rhs_of(h):
                        s = 2 * j + h          # 512-col strip index, 0..7
                        if s < 2:
                            return xg[s][:, 0:512]
                        return xg[2 + (s - 2) // 2][:, ((s - 2) % 2) * 512:
                                                    ((s - 2) % 2) * 512 + 512]
                else:
                    rhs_of = lambda h: xg[:, j * BT + h * 512:
                                          j * BT + (h + 1) * 512]
                ps = ps12pool.tile([128, BT], f32, tag="ps12")
                mm_pair(ps, w1_of(c), rhs_of)
                ps1[t] = ps

            def s1act(t):
                bt, c = divmod(t, NCHUNK)
                ps = ps1.pop(t)
                h1 = h1pool.tile([128, BT], bf16, tag="h1")
                if t in scalar_s1:
                    # psum holds kappa*(z+b1); tanh(z/2) = 2*(sigmoid(z)-.5)
                    # matches the centered convention of the DVE poly path.
                    nc.scalar.activation(h1[:], ps[:], TANH,
                                         scale=float(0.5 / _KAPPA))
                else:
                    nc.vector._custom_dve(
                        sig_op, out=h1[:], in0=ps[:],
                        s0=float(_CLAMP), s1=float(_ALPHA),
                        imm2=float(_BETA),
                    )
                h1d[t] = h1

            def s2(t):
                bt, c = divmod(t, NCHUNK)
                ps = ps12pool.tile([128, BT], f32, tag="ps12")
                h1 = h1d.pop(t)
                mm_pair(ps, w2_of(c),
                        lambda h: h1[:, h * 512:(h + 1) * 512])
                h2 = h2pool.tile([128, BT], bf16, tag="h2")
                # psum holds kappa*(z2 - b2'); h2 is centered 2*(sigmoid-.5)
                # (the /2 and +.5 are folded into W3/b3 on the host)
                if t >= NITER - 4:
                    # drain phase: ScalarE is the long pole, VectorE idle
                    nc.vector._custom_dve(
                        sig2_op, out=h2[:], in0=ps[:],
                        s0=b2kb[:, c:c + 1], s1=float(_ALPHA),
                        imm2=float(_BETA),
                    )
                else:
                    nc.scalar.activation(h2[:], ps[:], TANH,
                                         scale=float(0.5 / _KAPPA),
                                         bias=b2sb[:, c:c + 1])
                h2d[t] = h2

            def s3(t):
                bt, c = divmod(t, NCHUNK)
                if c == 0:
                    ps3[bt] = ps3pool.tile([16, BT], f32, tag="ps3", name=f"ps3_{bt}")
                h2 = h2d.pop(t)
                for h in range(2):
                    mm = nc.tensor.matmul(
                        ps3[bt][:, h * 512:(h + 1) * 512],
                        lhsT=w3sb[:, c * 16:(c + 1) * 16],
                        rhs=h2[:, h * 512:(h + 1) * 512],
                        start=(c == 0), stop=(c == NCHUNK - 1),
                    )
                    if h == 1:
                        mm.ins.ldweights = False
                if c == NCHUNK - 1:
                    osb = opool.tile([16, BT], f32, tag="osb")
                    nc.vector.tensor_scalar_add(osb[:], ps3[bt][:],
                                                b3sb[:, 0:1])
                    nc.sync.dma_start(out_ext[:, bt * BT:(bt + 1) * BT],
                                      osb[:])

            # Software pipeline: stage1 runs 2 iterations ahead of stage2 and
            # stage3 trails one behind.  ScalarE's stream alternates
            # s1act(t+1) (for its share of tiles), s2-act(t); VectorE drains
            # the other stage-1 tiles in parallel from different PSUM banks.
            s1mm(0)
            s1mm(1)
            s1act(0)
            for t in range(NITER):
                load_late_weights(t)
                # keep the x prefetch ~1 tile (4 iterations) ahead of the
                # consuming s1mm so a ~3us tile DMA never stalls the PE
                g4 = (t + 5) // 4
                if t + 5 < NITER and g4 not in xt:
                    load_x(g4)
                if t + 2 < NITER:
                    s1mm(t + 2)
                if t + 1 < NITER:
                    s1act(t + 1)
                s2(t)
                if t >= 1:
                    s3(t - 1)
            s3(NITER - 1)

    nc.compile()
    return nc


def _get_graph():
    if "nc" not in _graph_cache:
        _graph_cache["nc"] = _build_graph()
    return _graph_cache["nc"]


def _bias_lag(causal):
    """Per group, the first masked lag (its x channel is free to carry the
    constant-1 bias input).  Returns None if some group has no masked lag
    (probability ~2^-16 per group; the caller must then fall back)."""
    mask = (np.asarray(causal).reshape(G, L) != 0)
    if mask.all(axis=1).any():
        return None
    return np.argmin(mask, axis=1)          # first False per group


def _prep_shared(causal, W1, b1, W2, b2, W3, b3, l0):
    """Host-side weight packing (replicated across cores).

    Stage-1 weights are scaled by kappa and the bias b1 rides a spare
    all-zero (masked) weight row per group, paired with a constant-1 x
    channel -- so the activations need no per-partition bias operand.
    Stage-2 weights are halved and b2 shifted to consume the centered
    2*(sigmoid-0.5) convention of h1.
    """
    bf = ml_dtypes.bfloat16
    mask = (np.asarray(causal).reshape(G, L) != 0).astype(np.float32)
    W1m = np.asarray(W1, dtype=np.float32) * mask[:, None, :]   # [G, H, L]

    def blockdiag(blk):
        # blk: [G, K_in=16, M_out=16] -> [128 (gl*16+k), NCHUNK*128 (c*128+m)]
        bd = np.zeros((NCHUNK, GRP, 16, GRP, 16), dtype=np.float32)
        b5 = blk.reshape(NCHUNK, GRP, 16, 16)
        for gl in range(GRP):
            bd[:, gl, :, gl, :] = b5[:, gl]
        return np.ascontiguousarray(
            bd.reshape(NCHUNK, 128, 128).transpose(1, 0, 2).reshape(128, -1))

    w1blk = W1m.transpose(0, 2, 1) * _KAPPA                     # [G, L(k), H]
    w1blk[np.arange(G), l0, :] = np.asarray(b1, np.float32) * _KAPPA
    w1h = blockdiag(w1blk).astype(bf)
    # W2'' = kappa * W2 / 2: psum2 = kappa*(z2 - b2'); h2 comes out as the
    # centered 2*(sigmoid-0.5), consumed by halved W3 + shifted b3.
    w2h = blockdiag(
        np.asarray(W2, dtype=np.float32).transpose(0, 2, 1)
        * (0.5 * _KAPPA)).astype(bf)

    W3f = np.asarray(W3, dtype=np.float32) * 0.5                # [N, N*H]
    w3bd = np.zeros((NCHUNK, 128, 16), dtype=np.float32)
    for c in range(NCHUNK):
        w3bd[c, :, c // 2] = W3f[c // 2, (c % 2) * 128:(c % 2) * 128 + 128]
    w3h = np.ascontiguousarray(
        w3bd.transpose(1, 0, 2).reshape(128, NCHUNK * 16)).astype(bf)

    b2p = np.asarray(b2, np.float32) + 0.5 * np.asarray(W2, np.float32).sum(axis=2)
    b2s = np.ascontiguousarray((b2p * 0.5).reshape(NCHUNK, 128).T)
    b2k = np.ascontiguousarray((b2p * _KAPPA).reshape(NCHUNK, 128).T)
    b3p = (np.asarray(b3, dtype=np.float32)
           + 0.5 * np.asarray(W3, np.float32).sum(axis=1))
    b3h = np.ascontiguousarray(b3p.reshape(16, 1))
    return w1h, w2h, w3h, b2s, b2k, b3h


def _prep_x_shard(x_shard):
    """[BS, G, L] -> bricked channel-major [NBT, 8, 128, 4096] fp8e4m3."""
    xs = np.asarray(x_shard, dtype=np.float32).reshape(BS, C).T  # [C, BS]
    x5 = xs.reshape(8, 4, 128, NBT, BT)       # [cg, j, p, bt, col]
    return np.ascontiguousarray(x5.transpose(3, 0, 2, 1, 4)
                                .reshape(NBT, 8, 128, 4 * BT)
                                .astype(ml_dtypes.float8_e4m3fn))


def _run(inputs, trace=False, trace_cores=None):
    from concourse.bass_utils import run_bass_kernel_spmd

    l0 = _bias_lag(inputs["causal"])
    if l0 is None:
        raise RuntimeError(
            "some group has no masked lag; bias-in-matmul layout unavailable")
    nc = _get_graph()
    w1h, w2h, w3h, b2s, b2k, b3h = _prep_shared(
        inputs["causal"], inputs["W1"], inputs["b1"], inputs["W2"],
        inputs["b2"], inputs["W3"], inputs["b3"], l0)
    x = np.array(inputs["x"], dtype=np.float32)         # copy: mutated below
    # constant-1 bias input on each group's masked lag channel
    x.reshape(B, G * L)[:, np.arange(G) * L + l0] = 1.0
    in_maps = []
    for i in range(NCORES):
        in_maps.append({
            "x": _prep_x_shard(x[i * BS:(i + 1) * BS]),
            "w1": w1h, "w2": w2h, "w3": w3h,
            "b2s": b2s, "b2k": b2k, "b3": b3h,
        })
    res = None
    last_err = None
    for attempt in range(3):
        try:
            res = run_bass_kernel_spmd(
                nc, in_maps, list(range(NCORES)),
                trace=trace, trace_cores=trace_cores)
            break
        except Exception as e:  # transient NRT device wedge heals on rerun
            last_err = e
            time.sleep(2.0)
    if res is None:
        raise last_err
    out = np.empty((B, N), dtype=np.float32)
    for i in range(NCORES):
        out[i * BS:(i + 1) * BS] = res.results[i]["out"].T
    return out, res


def kernel(**inputs) -> np.ndarray:
    out, _ = _run(inputs, trace=False)
    return out



# revision 42
# speedup vs baseline: 1.0050x; 1.0050x over previous
"""Trainium2 Bass kernel for nn_AdditiveModel (grouped per-edge MLP + masked lag conv).

Reference computation (B=32768, N=16, L=16, H=16, G=N*N=256):
    xm  = x * (causal != 0)                     # [B, G, L]
    h1  = sigmoid(einsum('bgl,ghl->bgh', xm, W1) + b1)
    h2  = sigmoid(einsum('bgh,gkh->bgk', h1, W2) + b2)
    out = einsum('bvm,vm->bv', h2.reshape(B,N,N*H), W3) + b3   # [B, 16]

Strategy (pure data parallel over 8 NeuronCores, batch-sharded):
  - causal mask folded into W1 on the host; 8 groups packed block-diagonally
    per 128x128 weight tile so the per-group convs are full-width TensorE
    matmuls; W3 folded into a block-structured PSUM-accumulated stage 3.
  - The kernel is activation-bound: 2 sigmoids x 16.8M elements per core.
    ScalarE alone (1 elem/cycle/lane @1.2GHz + ~350cyc/instr overhead) costs
    ~294us.  To break that wall the sigmoid work is SPLIT between ScalarE and
    VectorE:
      * ScalarE: all stage-2 sigmoids + K_SCALAR of the 128 stage-1 tiles
        (exact, table-based ACT with fused bias).
      * VectorE: the remaining stage-1 tiles via two custom DVE ops:
          SIGEXP1P_ANT:  t = (cb - z/32)^32 + 1  ~= 1 + e^-(z+b1)
                         (FMA + 5 chained squarings + add = 8/8 ALU stages)
          RECIPROCAL_APPROX_FAST (stock): h1 = 1/t  (~51 ULP), bf16 out.
        The (1-u/32)^32 exponential underestimates by ~u^2/64 which perturbs
        h1 by <9e-3 absolute; filtered through W2 + sigmoid' this moves the
        final output rel-err from 1.88e-3 to only 1.91e-3 (verified vs
        reference) -- far inside the 2e-2 gate.  Custom ops are registered at
        import time (runtime uops-sha bootstrap), so kernel.py stays
        self-contained.
  - x and W1 are shipped bf16 (halves the dominant x DMA stream); stage-1
    matmul runs bf16 x bf16 at full PE rate.  h1/h2/W2/W3 bf16, PSUM fp32.
  - b1 rides on ScalarE's ACT bias; for DVE tiles it is pre-folded on the
    host into cb = 1 - b1/32 (per-partition scalar of SIGEXP1P_ANT).
  - x is host-bricked channel-major so every x DMA is fully contiguous.
  - the trace is software-pipelined (stage1 two iters ahead, stage3 one
    behind); the two act engines drain different PSUM banks in parallel.
"""

import sys
import time

import numpy as np

import ml_dtypes

if "/opt/trn_rl_repo" not in sys.path:
    sys.path.insert(0, "/opt/trn_rl_repo")

N = 16
L = 16
H = 16
B = 32768
G = N * N                 # 256 groups
NCORES = 8
BS = B // NCORES          # 4096 batch rows per core
C = G * L                 # 4096 channels (also G*H)
NCHUNK = 32               # channel chunks of 128
GRP = 8                   # groups per chunk
NBT = 4                   # batch tiles per core
BT = 1024                 # batch-tile width (columns)
NITER = NBT * NCHUNK      # 128 iterations per core

# stage-1 activation engine split: most stage-1 tiles go to VectorE via the
# single fused SIGPOLY5C op (~1.28us/tile, 1x PSUM stream); ScalarE keeps
# all 128 stage-2 sigmoids (~1.11us/tile) plus a small stage-1 share.
# Balancing (s + 128)*1.11 = (128 - s)*1.28 gives s ~= 10.
_S1_SCALAR_EVERY = 13      # t % 13 == 6 -> ScalarE (10 of 128 tiles)

# degree-5 odd minimax fit of sigmoid(z)-1/2 on [-A, A], A=4.25
# (max |z1| on the reference distribution is ~3.5; the clamp saturates
# gracefully for anything beyond).  kappa = 2*c1 folds the leading poly
# coefficient into the stage-1 weight/bias scale.
_POLY_A = 4.25
_POLY_C1 = 0.23677067
_POLY_C3 = -0.01223574
_POLY_C5 = 0.00030701
_KAPPA = 2 * _POLY_C1                       # 0.47354134
_CLAMP = _KAPPA * _POLY_A                   # s0
_ALPHA = _POLY_C3 / (_POLY_C1 * _KAPPA ** 2)        # s1
_BETA = _POLY_C5 / (_POLY_C1 * _KAPPA ** 4)         # imm2

_graph_cache = {}


def _register_dve_ops():
    """Register the custom SIGPOLY5C_ANT op (idempotent).

    One fused 8-stage pass computing a centered sigmoid approximation on
    the pre-scaled stage-1 preactivation v = kappa*(z + b1) (the scale and
    the bias both live in the matmul weights, so the op needs no
    per-partition operands):

        v  = min(max(in0, -s0), s0)          # clamp at kappa*A
        u  = v*v
        out = ((imm2*u + s1)*u + 1)*v        # = 2*(sigmoid(z+b1) - 0.5)

    With kappa = 2*c1 of the degree-5 odd minimax fit of sigmoid-1/2 on
    [-A, A], the leading coefficient folds to exactly 1.  uops_sha is
    bootstrapped at runtime so this works against any repo checkout.
    """
    from concourse import dve_ops
    from concourse.dve_spec import (
        Spec, Src0, C0, C1, C2, Latch, One, Zero, sq, lower, maxx, minn,
    )
    from concourse.dve_spec import _has_src1 as has_src1
    from concourse.dve_uop import DveOpSpec

    def _register(name, body, ref):
        if name in dve_ops._SUB_OPCODE_FOR_NAME:
            return next(o for o in dve_ops.OPS if o.name == name)
        op = dve_ops.DveOp(name, Spec(body=body, reference=ref),
                           subdim=False, uops_sha={})
        dve_ops.OPS.append(op)
        dve_ops._SUB_OPCODE_FOR_NAME[op.name] = (
            dve_ops._CUSTOM_DVE_ROW_BASE + len(dve_ops.OPS) - 1
        )
        dve_ops.CUSTOM_DVE_SPECS[op.name] = op.spec
        shas = {}
        for ver in ("v3", "v4"):
            spec_c = DveOpSpec(
                name=op.name,
                opcode=dve_ops.get_dve_sub_opcode(op.name),
                uops=lower(op.spec, ver=ver),
                rd1_en=has_src1(op.spec),
            )
            shas[ver] = spec_c.sha(ver)
        object.__setattr__(op, "uops_sha", shas)
        return op

    def _ref_c(in0, in1, s0, s1, imm2):
        v = np.clip(in0.astype(np.float32), -s0, s0)
        u = (v * v).astype(np.float32)
        return (((imm2 * u + s1) * u + np.float32(1.0)) * v).astype(np.float32)

    _v = maxx(minn(Src0, C0), Latch(Zero - C0))
    _u = sq(_v)
    op_c = _register(
        "SIGPOLY5C_ANT", ((C2 * _u + C1) * _u + One) * _v, _ref_c)

    # Bias-carrying clampless variant for stage-2 tiles (|kappa*z2| is
    # bounded ~0.64 << kappa*A on this model, so no clamp is needed and
    # the freed const slot carries the per-partition bias kappa*b2').
    def _ref_b(in0, in1, s0, s1, imm2):
        v = (in0.astype(np.float32) + s0).astype(np.float32)
        u = (v * v).astype(np.float32)
        return (((imm2 * u + s1) * u + np.float32(1.0)) * v).astype(np.float32)

    _vb = Src0 + C0
    _ub = sq(_vb)
    op_b = _register(
        "SIGPOLY5B_ANT", ((C2 * _ub + C1) * _ub + One) * _vb, _ref_b)
    return op_c, op_b


def _scalar_s1_iters():
    """ScalarE-owned stage-1 tiles (small share; see balance note above).
    None near the tail: the drain phase is ScalarE-bound, so the last
    stage-1 tiles stay on the (by then idle) VectorE."""
    return {t for t in range(NITER) if (t % _S1_SCALAR_EVERY) == 6 and t < 110}


def _build_graph():
    """Build + compile the per-core Bass graph (shared SPMD across 8 cores)."""
    from concourse import bacc, tile, mybir

    sig_op, sig2_op = _register_dve_ops()

    f32 = mybir.dt.float32
    bf16 = mybir.dt.bfloat16
    fp8 = mybir.dt.float8e4
    SIG = mybir.ActivationFunctionType.Sigmoid
    TANH = mybir.ActivationFunctionType.Tanh

    nc = bacc.Bacc("TRN2", target_bir_lowering=False, debug=False,
                   num_devices=NCORES)

    # x: [bt, cg, p, (j, col)] -- host-bricked so each [128, 4096] tile is a
    # single fully-contiguous 512 KiB DMA.  8 column-groups (cg) of 4 chunks
    # (j).  fp8e4m3: halves the dominant HBM stream vs bf16; the x
    # quantization error (~3% rel) perturbs h1 by <4e-3 which is far inside
    # the 2e-2 gate (verified).
    x_ext = nc.declare_dram_parameter("x", [NBT, 8, 128, 4096], fp8, isOutput=False)
    w1_ext = nc.declare_dram_parameter("w1", [128, NCHUNK * 128], bf16, isOutput=False)
    w2_ext = nc.declare_dram_parameter("w2", [128, NCHUNK * 128], bf16, isOutput=False)
    w3_ext = nc.declare_dram_parameter("w3", [128, NCHUNK * 16], bf16, isOutput=False)
    b2s_ext = nc.declare_dram_parameter("b2s", [128, NCHUNK], f32, isOutput=False)
    b2k_ext = nc.declare_dram_parameter("b2k", [128, NCHUNK], f32, isOutput=False)
    b3_ext = nc.declare_dram_parameter("b3", [16, 1], f32, isOutput=False)
    out_ext = nc.declare_dram_parameter("out", [16, BS], f32, isOutput=True)

    scalar_s1 = _scalar_s1_iters()

    with tile.TileContext(nc) as tc:
        with (
            tc.tile_pool(name="consts", bufs=1) as cpool,
            tc.tile_pool(name="xin", bufs=8) as xpool,
            tc.tile_pool(name="h1", bufs=4) as h1pool,
            tc.tile_pool(name="h2", bufs=4) as h2pool,
            tc.tile_pool(name="osb", bufs=2) as opool,
            tc.tile_pool(name="ps12", bufs=3, space="PSUM") as ps12pool,
            tc.tile_pool(name="ps3", bufs=1, space="PSUM") as ps3pool,
        ):
            xt = {}        # group idx -> x tile [128, 4096]

            def load_x_early(g):
                gbt, cg = divmod(g, 8)
                t = xpool.tile([128, 4096], fp8, tag="xin", name=f"x_{g}")
                nc.sync.dma_start(t[:], x_ext[gbt, cg])
                xt[g] = t

            # warm the sigmoid ACT table while the first DMAs stream
            warmsrc = cpool.tile([128, 1], f32)
            nc.vector.memset(warmsrc[:], 0.0)
            warm = cpool.tile([128, 1], f32)
            nc.scalar.activation(warm[:], warmsrc[:], SIG)
            # HAM warm-up: the PE clock gate defaults to K=4/8 (1.2 GHz) and
            # only releases after ~3.4us of sustained PE activity.  The real
            # pipeline can't start until the first x strip lands (~11us), so
            # without this the first ~28us of matmuls run at half clock
            # (measured).  Keep the PE busy with small dummy matmuls on a
            # zeroed tile while the DMAs stream; by the time real data
            # arrives the array is at 2.4 GHz.
            wsrc = cpool.tile([128, 128], bf16, name="warm_mm_src")
            nc.vector.memset(wsrc[:], 0.0)
            wps = ps3pool.tile([16, 128], f32, tag="ps3", name="warm_mm_ps")
            for _ in range(40):
                nc.tensor.matmul(wps[:], lhsT=wsrc[:, 0:16], rhs=wsrc[:],
                                 start=True, stop=True)
            # ramp: the very first matmul is gated only by w1 chunk 0
            # ([128,128]) and the first 512-col x strip; everything else
            # streams behind it.
            # Weight/bias streams ride the GpSimd DMA queue: each
            # DMA_DIRECT2D trigger costs ~0.7us on its issuing queue, and
            # keeping the Sync queue exclusively for x tiles lets the x
            # prefetch issue without queuing behind weight transfers.
            w1p = [cpool.tile([128, 8 * 128], bf16, name=f"w1p{i}")
                   for i in range(4)]
            w2p = [cpool.tile([128, 8 * 128], bf16, name=f"w2p{i}")
                   for i in range(4)]
            nc.gpsimd.dma_start(w1p[0][:, 0:128], w1_ext[:, 0:128])
            x0s = []
            for s in range(2):
                xp = xpool.tile([128, 512], fp8, tag="xin", name=f"x0s{s}")
                nc.sync.dma_start(xp[:], x_ext[0, 0, :, s * 512:(s + 1) * 512])
                x0s.append(xp)
            nc.gpsimd.dma_start(w1p[0][:, 128:1024], w1_ext[:, 128:1024])
            nc.gpsimd.dma_start(w2p[0][:], w2_ext[:, 0:1024])
            b2sb = cpool.tile([128, NCHUNK], f32)
            nc.gpsimd.dma_start(b2sb[:], b2s_ext[:])
            b2kb = cpool.tile([128, NCHUNK], f32)
            nc.gpsimd.dma_start(b2kb[:], b2k_ext[:])
            x0_parts = list(x0s)
            for j in range(1, 4):
                xp = xpool.tile([128, 1024], fp8, tag="xin", name=f"x0_{j}")
                nc.sync.dma_start(xp[:], x_ext[0, 0, :, j * 1024:(j + 1) * 1024])
                x0_parts.append(xp)
            xt[0] = x0_parts
            load_x_early(1)
            w3sb = cpool.tile([128, NCHUNK * 16], bf16)
            nc.gpsimd.dma_start(w3sb[:], w3_ext[:])
            b3sb = cpool.tile([16, 1], f32)
            nc.gpsimd.dma_start(b3sb[:], b3_ext[:])

            def w1_of(c):
                return w1p[c // 8][:, (c % 8) * 128:(c % 8 + 1) * 128]

            def w2_of(c):
                return w2p[c // 8][:, (c % 8) * 128:(c % 8 + 1) * 128]

            def load_late_weights(t):
                if t in (1, 2, 3):
                    nc.gpsimd.dma_start(w1p[t][:],
                                        w1_ext[:, t * 1024:(t + 1) * 1024])
                elif t in (4, 5, 6):
                    i = t - 3
                    nc.gpsimd.dma_start(w2p[i][:],
                                        w2_ext[:, i * 1024:(i + 1) * 1024])
            ps1 = {}
            h1d = {}
            h2d = {}
            ps3 = [None] * NBT

            load_x = load_x_early

            def mm_pair(ps, lhsT, rhs_of, start=True, stop=True):
                # second matmul of the pair reuses the PE-resident stationary
                # weights (skips its LDWEIGHTS)
                for h in range(2):
                    mm = nc.tensor.matmul(
                        ps[:, h * 512:(h + 1) * 512],
                        lhsT=lhsT,
                        rhs=rhs_of(h),
                        start=start, stop=stop,
                    )
                    if h == 1:
                        mm.ins.ldweights = False

            def s1mm(t):
                bt, c = divmod(t, NCHUNK)
                g, j = divmod(t, 4)
                if g not in xt:
                    load_x(g)
                xg = xt[g]
                if isinstance(xg, list):
                    def rhs_of(h):
                        s = 2 * j + h          # 512-col strip index, 0..7
                        if s < 2:
                            return xg[s][:, 0:512]
                        return xg[2 + (s - 2) // 2][:, ((s - 2) % 2) * 512:
                                                    ((s - 2) % 2) * 512 + 512]
                else:
                    rhs_of = lambda h: xg[:, j * BT + h * 512:
                                          j * BT + (h + 1) * 512]
                ps = ps12pool.tile([128, BT], f32, tag="ps12")
                mm_pair(ps, w1_of(c), rhs_of)
                ps1[t] = ps

            def s1act(t):
                bt, c = divmod(t, NCHUNK)
                ps = ps1.pop(t)
                h1 = h1pool.tile([128, BT], bf16, tag="h1")
                if t in scalar_s1:
                    # psum holds kappa*(z+b1); tanh(z/2) = 2*(sigmoid(z)-.5)
                    # matches the centered convention of the DVE poly path.
                    nc.scalar.activation(h1[:], ps[:], TANH,
                                         scale=float(0.5 / _KAPPA))
                else:
                    nc.vector._custom_dve(
                        sig_op, out=h1[:], in0=ps[:],
                        s0=float(_CLAMP), s1=float(_ALPHA),
                        imm2=float(_BETA),
                    )
                h1d[t] = h1

            def s2(t):
                bt, c = divmod(t, NCHUNK)
                ps = ps12pool.tile([128, BT], f32, tag="ps12")
                h1 = h1d.pop(t)
                mm_pair(ps, w2_of(c),
                        lambda h: h1[:, h * 512:(h + 1) * 512])
                h2 = h2pool.tile([128, BT], bf16, tag="h2")
                # psum holds kappa*(z2 - b2'); h2 is centered 2*(sigmoid-.5)
                # (the /2 and +.5 are folded into W3/b3 on the host)
                if t >= NITER - 4 and t % 2 == 0:
                    # drain phase: ScalarE is the long pole, VectorE idle
                    nc.vector._custom_dve(
                        sig2_op, out=h2[:], in0=ps[:],
                        s0=b2kb[:, c:c + 1], s1=float(_ALPHA),
                        imm2=float(_BETA),
                    )
                else:
                    nc.scalar.activation(h2[:], ps[:], TANH,
                                         scale=float(0.5 / _KAPPA),
                                         bias=b2sb[:, c:c + 1])
                h2d[t] = h2

            def s3(t):
                bt, c = divmod(t, NCHUNK)
                if c == 0:
                    ps3[bt] = ps3pool.tile([16, BT], f32, tag="ps3", name=f"ps3_{bt}")
                h2 = h2d.pop(t)
                for h in range(2):
                    mm = nc.tensor.matmul(
                        ps3[bt][:, h * 512:(h + 1) * 512],
                        lhsT=w3sb[:, c * 16:(c + 1) * 16],
                        rhs=h2[:, h * 512:(h + 1) * 512],
                        start=(c == 0), stop=(c == NCHUNK - 1),
                    )
                    if h == 1:
                        mm.ins.ldweights = False
                if c == NCHUNK - 1:
                    osb = opool.tile([16, BT], f32, tag="osb")
                    nc.vector.tensor_scalar_add(osb[:], ps3[bt][:],
                                                b3sb[:, 0:1])
                    nc.sync.dma_start(out_ext[:, bt * BT:(bt + 1) * BT],
                                      osb[:])

            # Software pipeline: stage1 runs 2 iterations ahead of stage2 and
            # stage3 trails one behind.  ScalarE's stream alternates
            # s1act(t+1) (for its share of tiles), s2-act(t); VectorE drains
            # the other stage-1 tiles in parallel from different PSUM banks.
            s1mm(0)
            s1mm(1)
            s1act(0)
            for t in range(NITER):
                load_late_weights(t)
                # keep the x prefetch ~1 tile (4 iterations) ahead of the
                # consuming s1mm so a ~3us tile DMA never stalls the PE
                g4 = (t + 5) // 4
                if t + 5 < NITER and g4 not in xt:
                    load_x(g4)
                if t + 2 < NITER:
                    s1mm(t + 2)
                if t + 1 < NITER:
                    s1act(t + 1)
                s2(t)
                if t >= 1:
                    s3(t - 1)
            s3(NITER - 1)

    nc.compile()
    return nc


def _get_graph():
    if "nc" not in _graph_cache:
        _graph_cache["nc"] = _build_graph()
    return _graph_cache["nc"]


def _bias_lag(causal):
    """Per group, the first masked lag (its x channel is free to carry the
    constant-1 bias input).  Returns None if some group has no masked lag
    (probability ~2^-16 per group; the caller must then fall back)."""
    mask = (np.asarray(causal).reshape(G, L) != 0)
    if mask.all(axis=1).any():
        return None
    return np.argmin(mask, axis=1)          # first False per group


def _prep_shared(causal, W1, b1, W2, b2, W3, b3, l0):
    """Host-side weight packing (replicated across cores).

    Stage-1 weights are scaled by kappa and the bias b1 rides a spare
    all-zero (masked) weight row per group, paired with a constant-1 x
    channel -- so the activations need no per-partition bias operand.
    Stage-2 weights are halved and b2 shifted to consume the centered
    2*(sigmoid-0.5) convention of h1.
    """
    bf = ml_dtypes.bfloat16
    mask = (np.asarray(causal).reshape(G, L) != 0).astype(np.float32)
    W1m = np.asarray(W1, dtype=np.float32) * mask[:, None, :]   # [G, H, L]

    def blockdiag(blk):
        # blk: [G, K_in=16, M_out=16] -> [128 (gl*16+k), NCHUNK*128 (c*128+m)]
        bd = np.zeros((NCHUNK, GRP, 16, GRP, 16), dtype=np.float32)
        b5 = blk.reshape(NCHUNK, GRP, 16, 16)
        for gl in range(GRP):
            bd[:, gl, :, gl, :] = b5[:, gl]
        return np.ascontiguousarray(
            bd.reshape(NCHUNK, 128, 128).transpose(1, 0, 2).reshape(128, -1))

    w1blk = W1m.transpose(0, 2, 1) * _KAPPA                     # [G, L(k), H]
    w1blk[np.arange(G), l0, :] = np.asarray(b1, np.float32) * _KAPPA
    w1h = blockdiag(w1blk).astype(bf)
    # W2'' = kappa * W2 / 2: psum2 = kappa*(z2 - b2'); h2 comes out as the
    # centered 2*(sigmoid-0.5), consumed by halved W3 + shifted b3.
    w2h = blockdiag(
        np.asarray(W2, dtype=np.float32).transpose(0, 2, 1)
        * (0.5 * _KAPPA)).astype(bf)

    W3f = np.asarray(W3, dtype=np.float32) * 0.5                # [N, N*H]
    w3bd = np.zeros((NCHUNK, 128, 16), dtype=np.float32)
    for c in range(NCHUNK):
        w3bd[c, :, c // 2] = W3f[c // 2, (c % 2) * 128:(c % 2) * 128 + 128]
    w3h = np.ascontiguousarray(
        w3bd.transpose(1, 0, 2).reshape(128, NCHUNK * 16)).astype(bf)

    b2p = np.asarray(b2, np.float32) + 0.5 * np.asarray(W2, np.float32).sum(axis=2)
    b2s = np.ascontiguousarray((b2p * 0.5).reshape(NCHUNK, 128).T)
    b2k = np.ascontiguousarray((b2p * _KAPPA).reshape(NCHUNK, 128).T)
    b3p = (np.asarray(b3, dtype=np.float32)
           + 0.5 * np.asarray(W3, np.float32).sum(axis=1))
    b3h = np.ascontiguousarray(b3p.reshape(16, 1))
    return w1h, w2h, w3h, b2s, b2k, b3h


def _prep_x_shard(x_shard):
    """[BS, G, L] -> bricked channel-major [NBT, 8, 128, 4096] fp8e4m3."""
    xs = np.asarray(x_shard, dtype=np.float32).reshape(BS, C).T  # [C, BS]
    x5 = xs.reshape(8, 4, 128, NBT, BT)       # [cg, j, p, bt, col]
    return np.ascontiguousarray(x5.transpose(3, 0, 2, 1, 4)
                                .reshape(NBT, 8, 128, 4 * BT)
                                .astype(ml_dtypes.float8_e4m3fn))


def _run(inputs, trace=False, trace_cores=None):
    from concourse.bass_utils import run_bass_kernel_spmd

    l0 = _bias_lag(inputs["causal"])
    if l0 is None:
        raise RuntimeError(
            "some group has no masked lag; bias-in-matmul layout unavailable")
    nc = _get_graph()
    w1h, w2h, w3h, b2s, b2k, b3h = _prep_shared(
        inputs["causal"], inputs["W1"], inputs["b1"], inputs["W2"],
        inputs["b2"], inputs["W3"], inputs["b3"], l0)
    x = np.array(inputs["x"], dtype=np.float32)         # copy: mutated below
    # constant-1 bias input on each group's masked lag channel
    x.reshape(B, G * L)[:, np.arange(G) * L + l0] = 1.0
    in_maps = []
    for i in range(NCORES):
        in_maps.append({
            "x": _prep_x_shard(x[i * BS:(i + 1) * BS]),
            "w1": w1h, "w2": w2h, "w3": w3h,
            "b2s": b2s, "b2k": b2k, "b3": b3h,
        })
    res = None
    last_err = None
    for attempt in range(3):
        try:
            res = run_bass_kernel_spmd(
                nc, in_maps, list(range(NCORES)),
                trace=trace, trace_cores=trace_cores)
            break
        except Exception as e:  # transient NRT device wedge heals on rerun
            last_err = e
            time.sleep(2.0)
    if res is None:
        raise last_err
    out = np.empty((B, N), dtype=np.float32)
    for i in range(NCORES):
        out[i * BS:(i + 1) * BS] = res.results[i]["out"].T
    return out, res


def kernel(**inputs) -> np.ndarray:
    out, _ = _run(inputs, trace=False)
    return out



# revision 46
# speedup vs baseline: 1.0151x; 1.0101x over previous
"""Trainium2 Bass kernel for nn_AdditiveModel (grouped per-edge MLP + masked lag conv).

Reference computation (B=32768, N=16, L=16, H=16, G=N*N=256):
    xm  = x * (causal != 0)                     # [B, G, L]
    h1  = sigmoid(einsum('bgl,ghl->bgh', xm, W1) + b1)
    h2  = sigmoid(einsum('bgh,gkh->bgk', h1, W2) + b2)
    out = einsum('bvm,vm->bv', h2.reshape(B,N,N*H), W3) + b3   # [B, 16]

Strategy (pure data parallel over 8 NeuronCores, batch-sharded):
  - causal mask folded into W1 on the host; 8 groups packed block-diagonally
    per 128x128 weight tile so the per-group convs are full-width TensorE
    matmuls; W3 folded into a block-structured PSUM-accumulated stage 3.
  - All custom DVE ops stream strictly 1 elem/cycle/lane (FD-linear,
    measured), and ScalarE ACTIVATE costs ~1.11us per [128,1024] PSUM tile.
    The activation wall is broken by making every sigmoid a SINGLE pass:
      * b1 rides the stage-1 MATMUL itself: every group has at least one
        causally-masked lag whose weight row is all-zero, so that row gets
        b1*kappa and its (dead) x channel is overwritten with 1.0 on the
        host.  No activation ever needs a bias operand for stage 1.
      * Stage-1 weights are pre-scaled by kappa = 2*c1 of a degree-5 odd
        minimax fit of sigmoid-1/2 on [-4.25, 4.25]; SIGPOLY5C_ANT then
        computes the centered sigmoid in ONE fused 8-stage DVE op
        (clamp via minn/maxx + latch-initialised -clamp, Horner poly).
      * ScalarE's stage-1 share uses tanh(z/2) (same ACT table set as
        sigmoid) which equals the same centered convention exactly.
      * Stage-2 runs on pre-scaled weights too (kappa*W2/2); ScalarE tiles
        use tanh with per-partition bias b2'/2, tail tiles use the
        clampless bias-carrying SIGPOLY5B_ANT on the otherwise-idle DVE.
        h2 is centered, with the /2 and +0.5 folded into W3 and b3.
    Custom ops are registered at import time (runtime uops-sha bootstrap),
    so kernel.py stays self-contained.  End-to-end rel-err 1.6e-3 vs the
    2e-2 gate.
  - x is shipped fp8e4m3 (quarter of the fp32 HBM stream; ~3% relative
    quantization perturbs h1 by <4e-3), weights bf16, PSUM fp32.
  - x is host-bricked channel-major so every x DMA is fully contiguous;
    weight/bias DMAs ride the GpSimd queue so the Sync queue is x-only.
  - ~40 tiny warm-up matmuls run during the DMA ramp so the PE HAM clock
    gate (default K=4/8 = 1.2GHz, releases only after ~3.4us of sustained
    activity) is at 2.4GHz when real data lands.
  - the trace is software-pipelined (stage1 two iters ahead, stage3 one
    behind); the two act engines drain different PSUM banks in parallel.
    After these changes the kernel is TensorE-bound (~87% PE occupancy,
    matmul issue gap at the 512-col streaming floor).
"""

import sys
import time

import numpy as np

import ml_dtypes

if "/opt/trn_rl_repo" not in sys.path:
    sys.path.insert(0, "/opt/trn_rl_repo")

N = 16
L = 16
H = 16
B = 32768
G = N * N                 # 256 groups
NCORES = 8
BS = B // NCORES          # 4096 batch rows per core
C = G * L                 # 4096 channels (also G*H)
NCHUNK = 32               # channel chunks of 128
GRP = 8                   # groups per chunk
NBT = 4                   # batch tiles per core
BT = 1024                 # batch-tile width (columns)
NITER = NBT * NCHUNK      # 128 iterations per core

# stage-1 activation engine split: most stage-1 tiles go to VectorE via the
# single fused SIGPOLY5C op (~1.28us/tile, 1x PSUM stream); ScalarE keeps
# all 128 stage-2 sigmoids (~1.11us/tile) plus a small stage-1 share.
# Balancing (s + 128)*1.11 = (128 - s)*1.28 gives s ~= 10.
_S1_SCALAR_EVERY = 13      # t % 13 == 6 -> ScalarE (10 of 128 tiles)

# degree-5 odd minimax fit of sigmoid(z)-1/2 on [-A, A], A=4.25
# (max |z1| on the reference distribution is ~3.5; the clamp saturates
# gracefully for anything beyond).  kappa = 2*c1 folds the leading poly
# coefficient into the stage-1 weight/bias scale.
_POLY_A = 4.25
_POLY_C1 = 0.23677067
_POLY_C3 = -0.01223574
_POLY_C5 = 0.00030701
_KAPPA = 2 * _POLY_C1                       # 0.47354134
_CLAMP = _KAPPA * _POLY_A                   # s0
_ALPHA = _POLY_C3 / (_POLY_C1 * _KAPPA ** 2)        # s1
_BETA = _POLY_C5 / (_POLY_C1 * _KAPPA ** 4)         # imm2

_graph_cache = {}


def _register_dve_ops():
    """Register the custom SIGPOLY5C_ANT op (idempotent).

    One fused 8-stage pass computing a centered sigmoid approximation on
    the pre-scaled stage-1 preactivation v = kappa*(z + b1) (the scale and
    the bias both live in the matmul weights, so the op needs no
    per-partition operands):

        v  = min(max(in0, -s0), s0)          # clamp at kappa*A
        u  = v*v
        out = ((imm2*u + s1)*u + 1)*v        # = 2*(sigmoid(z+b1) - 0.5)

    With kappa = 2*c1 of the degree-5 odd minimax fit of sigmoid-1/2 on
    [-A, A], the leading coefficient folds to exactly 1.  uops_sha is
    bootstrapped at runtime so this works against any repo checkout.
    """
    from concourse import dve_ops
    from concourse.dve_spec import (
        Spec, Src0, C0, C1, C2, Latch, One, Zero, sq, lower, maxx, minn,
    )
    from concourse.dve_spec import _has_src1 as has_src1
    from concourse.dve_uop import DveOpSpec

    def _register(name, body, ref):
        if name in dve_ops._SUB_OPCODE_FOR_NAME:
            return next(o for o in dve_ops.OPS if o.name == name)
        op = dve_ops.DveOp(name, Spec(body=body, reference=ref),
                           subdim=False, uops_sha={})
        dve_ops.OPS.append(op)
        dve_ops._SUB_OPCODE_FOR_NAME[op.name] = (
            dve_ops._CUSTOM_DVE_ROW_BASE + len(dve_ops.OPS) - 1
        )
        dve_ops.CUSTOM_DVE_SPECS[op.name] = op.spec
        shas = {}
        for ver in ("v3", "v4"):
            spec_c = DveOpSpec(
                name=op.name,
                opcode=dve_ops.get_dve_sub_opcode(op.name),
                uops=lower(op.spec, ver=ver),
                rd1_en=has_src1(op.spec),
            )
            shas[ver] = spec_c.sha(ver)
        object.__setattr__(op, "uops_sha", shas)
        return op

    def _ref_c(in0, in1, s0, s1, imm2):
        v = np.clip(in0.astype(np.float32), -s0, s0)
        u = (v * v).astype(np.float32)
        return (((imm2 * u + s1) * u + np.float32(1.0)) * v).astype(np.float32)

    _v = maxx(minn(Src0, C0), Latch(Zero - C0))
    _u = sq(_v)
    op_c = _register(
        "SIGPOLY5C_ANT", ((C2 * _u + C1) * _u + One) * _v, _ref_c)

    # Bias-carrying clampless variant for stage-2 tiles (|kappa*z2| is
    # bounded ~0.64 << kappa*A on this model, so no clamp is needed and
    # the freed const slot carries the per-partition bias kappa*b2').
    def _ref_b(in0, in1, s0, s1, imm2):
        v = (in0.astype(np.float32) + s0).astype(np.float32)
        u = (v * v).astype(np.float32)
        return (((imm2 * u + s1) * u + np.float32(1.0)) * v).astype(np.float32)

    _vb = Src0 + C0
    _ub = sq(_vb)
    op_b = _register(
        "SIGPOLY5B_ANT", ((C2 * _ub + C1) * _ub + One) * _vb, _ref_b)
    return op_c, op_b


def _scalar_s1_iters():
    """ScalarE-owned stage-1 tiles (small share; see balance note above).
    None near the tail: the drain phase is ScalarE-bound, so the last
    stage-1 tiles stay on the (by then idle) VectorE."""
    return {t for t in range(NITER) if (t % _S1_SCALAR_EVERY) == 6 and t < 110}


def _build_graph():
    """Build + compile the per-core Bass graph (shared SPMD across 8 cores)."""
    from concourse import bacc, tile, mybir

    sig_op, sig2_op = _register_dve_ops()

    f32 = mybir.dt.float32
    bf16 = mybir.dt.bfloat16
    fp8 = mybir.dt.float8e4
    SIG = mybir.ActivationFunctionType.Sigmoid
    TANH = mybir.ActivationFunctionType.Tanh
    IDENT = mybir.ActivationFunctionType.Identity

    nc = bacc.Bacc("TRN2", target_bir_lowering=False, debug=False,
                   num_devices=NCORES)

    # x: [bt, cg, p, (j, col)] -- host-bricked so each [128, 4096] tile is a
    # single fully-contiguous 512 KiB DMA.  8 column-groups (cg) of 4 chunks
    # (j).  fp8e4m3: halves the dominant HBM stream vs bf16; the x
    # quantization error (~3% rel) perturbs h1 by <4e-3 which is far inside
    # the 2e-2 gate (verified).
    x_ext = nc.declare_dram_parameter("x", [NBT, 8, 128, 4096], fp8, isOutput=False)
    w1_ext = nc.declare_dram_parameter("w1", [128, NCHUNK * 128], bf16, isOutput=False)
    w2_ext = nc.declare_dram_parameter("w2", [128, NCHUNK * 128], bf16, isOutput=False)
    w3_ext = nc.declare_dram_parameter("w3", [128, NCHUNK * 16], bf16, isOutput=False)
    b2s_ext = nc.declare_dram_parameter("b2s", [128, NCHUNK], f32, isOutput=False)
    b2k_ext = nc.declare_dram_parameter("b2k", [128, NCHUNK], f32, isOutput=False)
    b3_ext = nc.declare_dram_parameter("b3", [16, 1], f32, isOutput=False)
    out_ext = nc.declare_dram_parameter("out", [16, BS], f32, isOutput=True)

    scalar_s1 = _scalar_s1_iters()

    with tile.TileContext(nc) as tc:
        with (
            tc.tile_pool(name="consts", bufs=1) as cpool,
            tc.tile_pool(name="xin", bufs=8) as xpool,
            tc.tile_pool(name="h1", bufs=4) as h1pool,
            tc.tile_pool(name="h2", bufs=4) as h2pool,
            tc.tile_pool(name="osb", bufs=2) as opool,
            tc.tile_pool(name="ps12", bufs=3, space="PSUM") as ps12pool,
            tc.tile_pool(name="ps3", bufs=1, space="PSUM") as ps3pool,
        ):
            xt = {}        # group idx -> x tile [128, 4096]

            def load_x_early(g):
                gbt, cg = divmod(g, 8)
                t = xpool.tile([128, 4096], fp8, tag="xin", name=f"x_{g}")
                nc.sync.dma_start(t[:], x_ext[gbt, cg])
                xt[g] = t

            # warm the sigmoid ACT table while the first DMAs stream
            warmsrc = cpool.tile([128, 1], f32)
            nc.vector.memset(warmsrc[:], 0.0)
            warm = cpool.tile([128, 1], f32)
            nc.scalar.activation(warm[:], warmsrc[:], SIG)
            # HAM warm-up: the PE clock gate defaults to K=4/8 (1.2 GHz) and
            # only releases after ~3.4us of sustained PE activity.  The real
            # pipeline can't start until the first x strip lands (~11us), so
            # without this the first ~28us of matmuls run at half clock
            # (measured).  Keep the PE busy with small dummy matmuls on a
            # zeroed tile while the DMAs stream; by the time real data
            # arrives the array is at 2.4 GHz.
            wsrc = cpool.tile([128, 128], bf16, name="warm_mm_src")
            nc.vector.memset(wsrc[:], 0.0)
            wps = ps3pool.tile([16, 128], f32, tag="ps3", name="warm_mm_ps")
            for _ in range(40):
                nc.tensor.matmul(wps[:], lhsT=wsrc[:, 0:16], rhs=wsrc[:],
                                 start=True, stop=True)
            # ramp: the very first matmul is gated only by w1 chunk 0
            # ([128,128]) and the first 512-col x strip; everything else
            # streams behind it.
            # Weight/bias streams ride the GpSimd DMA queue: each
            # DMA_DIRECT2D trigger costs ~0.7us on its issuing queue, and
            # keeping the Sync queue exclusively for x tiles lets the x
            # prefetch issue without queuing behind weight transfers.
            w1p = [cpool.tile([128, 8 * 128], bf16, name=f"w1p{i}")
                   for i in range(4)]
            w2p = [cpool.tile([128, 8 * 128], bf16, name=f"w2p{i}")
                   for i in range(4)]
            nc.gpsimd.dma_start(w1p[0][:, 0:128], w1_ext[:, 0:128])
            x0s = []
            for s in range(2):
                xp = xpool.tile([128, 512], fp8, tag="xin", name=f"x0s{s}")
                nc.sync.dma_start(xp[:], x_ext[0, 0, :, s * 512:(s + 1) * 512])
                x0s.append(xp)
            nc.gpsimd.dma_start(w1p[0][:, 128:1024], w1_ext[:, 128:1024])
            nc.gpsimd.dma_start(w2p[0][:], w2_ext[:, 0:1024])
            b2sb = cpool.tile([128, NCHUNK], f32)
            nc.gpsimd.dma_start(b2sb[:], b2s_ext[:])
            b2kb = cpool.tile([128, NCHUNK], f32)
            nc.gpsimd.dma_start(b2kb[:], b2k_ext[:])
            x0_parts = list(x0s)
            for j in range(1, 4):
                xp = xpool.tile([128, 1024], fp8, tag="xin", name=f"x0_{j}")
                nc.sync.dma_start(xp[:], x_ext[0, 0, :, j * 1024:(j + 1) * 1024])
                x0_parts.append(xp)
            xt[0] = x0_parts
            load_x_early(1)
            w3sb = cpool.tile([128, NCHUNK * 16], bf16)
            nc.gpsimd.dma_start(w3sb[:], w3_ext[:])
            b3sb = cpool.tile([16, 1], f32)
            nc.gpsimd.dma_start(b3sb[:], b3_ext[:])

            def w1_of(c):
                return w1p[c // 8][:, (c % 8) * 128:(c % 8 + 1) * 128]

            def w2_of(c):
                return w2p[c // 8][:, (c % 8) * 128:(c % 8 + 1) * 128]

            def load_late_weights(t):
                if t in (1, 2, 3):
                    nc.gpsimd.dma_start(w1p[t][:],
                                        w1_ext[:, t * 1024:(t + 1) * 1024])
                elif t in (4, 5, 6):
                    i = t - 3
                    nc.gpsimd.dma_start(w2p[i][:],
                                        w2_ext[:, i * 1024:(i + 1) * 1024])
            ps1 = {}
            h1d = {}
            h2d = {}
            ps3 = [None] * NBT

            load_x = load_x_early

            def mm_pair(ps, lhsT, rhs_of, start=True, stop=True):
                # second matmul of the pair reuses the PE-resident stationary
                # weights (skips its LDWEIGHTS)
                for h in range(2):
                    mm = nc.tensor.matmul(
                        ps[:, h * 512:(h + 1) * 512],
                        lhsT=lhsT,
                        rhs=rhs_of(h),
                        start=start, stop=stop,
                    )
                    if h == 1:
                        mm.ins.ldweights = False

            def s1mm(t):
                bt, c = divmod(t, NCHUNK)
                g, j = divmod(t, 4)
                if g not in xt:
                    load_x(g)
                xg = xt[g]
                if isinstance(xg, list):
                    def rhs_of(h):
                        s = 2 * j + h          # 512-col strip index, 0..7
                        if s < 2:
                            return xg[s][:, 0:512]
                        return xg[2 + (s - 2) // 2][:, ((s - 2) % 2) * 512:
                                                    ((s - 2) % 2) * 512 + 512]
                else:
                    rhs_of = lambda h: xg[:, j * BT + h * 512:
                                          j * BT + (h + 1) * 512]
                ps = ps12pool.tile([128, BT], f32, tag="ps12")
                mm_pair(ps, w1_of(c), rhs_of)
                ps1[t] = ps

            def s1act(t):
                bt, c = divmod(t, NCHUNK)
                ps = ps1.pop(t)
                h1 = h1pool.tile([128, BT], bf16, tag="h1")
                if t in scalar_s1:
                    # psum holds kappa*(z+b1); tanh(z/2) = 2*(sigmoid(z)-.5)
                    # matches the centered convention of the DVE poly path.
                    nc.scalar.activation(h1[:], ps[:], TANH,
                                         scale=float(0.5 / _KAPPA))
                else:
                    nc.vector._custom_dve(
                        sig_op, out=h1[:], in0=ps[:],
                        s0=float(_CLAMP), s1=float(_ALPHA),
                        imm2=float(_BETA),
                    )
                h1d[t] = h1

            def s2(t):
                bt, c = divmod(t, NCHUNK)
                ps = ps12pool.tile([128, BT], f32, tag="ps12")
                h1 = h1d.pop(t)
                mm_pair(ps, w2_of(c),
                        lambda h: h1[:, h * 512:(h + 1) * 512])
                h2 = h2pool.tile([128, BT], bf16, tag="h2")
                # psum holds kappa*(z2 - b2'); h2 is centered 2*(sigmoid-.5)
                # (the /2 and +.5 are folded into W3/b3 on the host)
                if t >= NITER - 4 and t % 2 == 0:
                    # drain phase: ScalarE is the long pole, VectorE idle
                    nc.vector._custom_dve(
                        sig2_op, out=h2[:], in0=ps[:],
                        s0=b2kb[:, c:c + 1], s1=float(_ALPHA),
                        imm2=float(_BETA),
                    )
                else:
                    nc.scalar.activation(h2[:], ps[:], TANH,
                                         scale=float(0.5 / _KAPPA),
                                         bias=b2sb[:, c:c + 1])
                h2d[t] = h2

            def s3(t):
                bt, c = divmod(t, NCHUNK)
                if c == 0:
                    ps3[bt] = ps3pool.tile([16, BT], f32, tag="ps3", name=f"ps3_{bt}")
                h2 = h2d.pop(t)
                for h in range(2):
                    mm = nc.tensor.matmul(
                        ps3[bt][:, h * 512:(h + 1) * 512],
                        lhsT=w3sb[:, c * 16:(c + 1) * 16],
                        rhs=h2[:, h * 512:(h + 1) * 512],
                        start=(c == 0), stop=(c == NCHUNK - 1),
                    )
                    if h == 1:
                        mm.ins.ldweights = False
                if c == NCHUNK - 1:
                    # drain per 512-col half (per PSUM bank): the h=0 half is
                    # final as soon as the last h=0 matmul lands, so its
                    # Copy+bias (ScalarE -- idle in the drain phase; Copy is
                    # filler in every ACT table set, no table switch) and
                    # its DMA overlap the h=1 matmul still streaming.
                    osb = opool.tile([16, BT], f32, tag="osb")
                    for h in range(2):
                        nc.scalar.activation(
                            osb[:, h * 512:(h + 1) * 512],
                            ps3[bt][:, h * 512:(h + 1) * 512],
                            IDENT, bias=b3sb[:, 0:1])
                        nc.sync.dma_start(
                            out_ext[:, bt * BT + h * 512:
                                    bt * BT + (h + 1) * 512],
                            osb[:, h * 512:(h + 1) * 512])

            # Software pipeline: stage1 runs 2 iterations ahead of stage2 and
            # stage3 trails one behind.  ScalarE's stream alternates
            # s1act(t+1) (for its share of tiles), s2-act(t); VectorE drains
            # the other stage-1 tiles in parallel from different PSUM banks.
            s1mm(0)
            s1mm(1)
            s1act(0)
            for t in range(NITER):
                load_late_weights(t)
                # keep the x prefetch ~1 tile (4 iterations) ahead of the
                # consuming s1mm so a ~3us tile DMA never stalls the PE
                g4 = (t + 5) // 4
                if t + 5 < NITER and g4 not in xt:
                    load_x(g4)
                if t + 2 < NITER:
                    s1mm(t + 2)
                if t + 1 < NITER:
                    s1act(t + 1)
                s2(t)
                if t >= 1:
                    s3(t - 1)
            s3(NITER - 1)

    nc.compile()
    return nc


def _get_graph():
    if "nc" not in _graph_cache:
        _graph_cache["nc"] = _build_graph()
    return _graph_cache["nc"]


def _bias_lag(causal):
    """Per group, the first masked lag (its x channel is free to carry the
    constant-1 bias input).  Returns None if some group has no masked lag
    (probability ~2^-16 per group; the caller must then fall back)."""
    mask = (np.asarray(causal).reshape(G, L) != 0)
    if mask.all(axis=1).any():
        return None
    return np.argmin(mask, axis=1)          # first False per group


def _prep_shared(causal, W1, b1, W2, b2, W3, b3, l0):
    """Host-side weight packing (replicated across cores).

    Stage-1 weights are scaled by kappa and the bias b1 rides a spare
    all-zero (masked) weight row per group, paired with a constant-1 x
    channel -- so the activations need no per-partition bias operand.
    Stage-2 weights are halved and b2 shifted to consume the centered
    2*(sigmoid-0.5) convention of h1.
    """
    bf = ml_dtypes.bfloat16
    mask = (np.asarray(causal).reshape(G, L) != 0).astype(np.float32)
    W1m = np.asarray(W1, dtype=np.float32) * mask[:, None, :]   # [G, H, L]

    def blockdiag(blk):
        # blk: [G, K_in=16, M_out=16] -> [128 (gl*16+k), NCHUNK*128 (c*128+m)]
        bd = np.zeros((NCHUNK, GRP, 16, GRP, 16), dtype=np.float32)
        b5 = blk.reshape(NCHUNK, GRP, 16, 16)
        for gl in range(GRP):
            bd[:, gl, :, gl, :] = b5[:, gl]
        return np.ascontiguousarray(
            bd.reshape(NCHUNK, 128, 128).transpose(1, 0, 2).reshape(128, -1))

    w1blk = W1m.transpose(0, 2, 1) * _KAPPA                     # [G, L(k), H]
    w1blk[np.arange(G), l0, :] = np.asarray(b1, np.float32) * _KAPPA
    w1h = blockdiag(w1blk).astype(bf)
    # W2'' = kappa * W2 / 2: psum2 = kappa*(z2 - b2'); h2 comes out as the
    # centered 2*(sigmoid-0.5), consumed by halved W3 + shifted b3.
    w2h = blockdiag(
        np.asarray(W2, dtype=np.float32).transpose(0, 2, 1)
        * (0.5 * _KAPPA)).astype(bf)

    W3f = np.asarray(W3, dtype=np.float32) * 0.5                # [N, N*H]
    w3bd = np.zeros((NCHUNK, 128, 16), dtype=np.float32)
    for c in range(NCHUNK):
        w3bd[c, :, c // 2] = W3f[c // 2, (c % 2) * 128:(c % 2) * 128 + 128]
    w3h = np.ascontiguousarray(
        w3bd.transpose(1, 0, 2).reshape(128, NCHUNK * 16)).astype(bf)

    b2p = np.asarray(b2, np.float32) + 0.5 * np.asarray(W2, np.float32).sum(axis=2)
    b2s = np.ascontiguousarray((b2p * 0.5).reshape(NCHUNK, 128).T)
    b2k = np.ascontiguousarray((b2p * _KAPPA).reshape(NCHUNK, 128).T)
    b3p = (np.asarray(b3, dtype=np.float32)
           + 0.5 * np.asarray(W3, np.float32).sum(axis=1))
    b3h = np.ascontiguousarray(b3p.reshape(16, 1))
    return w1h, w2h, w3h, b2s, b2k, b3h


def _prep_x_shard(x_shard):
    """[BS, G, L] -> bricked channel-major [NBT, 8, 128, 4096] fp8e4m3."""
    xs = np.asarray(x_shard, dtype=np.float32).reshape(BS, C).T  # [C, BS]
    x5 = xs.reshape(8, 4, 128, NBT, BT)       # [cg, j, p, bt, col]
    return np.ascontiguousarray(x5.transpose(3, 0, 2, 1, 4)
                                .reshape(NBT, 8, 128, 4 * BT)
                                .astype(ml_dtypes.float8_e4m3fn))


def _run(inputs, trace=False, trace_cores=None):
    from concourse.bass_utils import run_bass_kernel_spmd

    l0 = _bias_lag(inputs["causal"])
    if l0 is None:
        raise RuntimeError(
            "some group has no masked lag; bias-in-matmul layout unavailable")
    nc = _get_graph()
    w1h, w2h, w3h, b2s, b2k, b3h = _prep_shared(
        inputs["causal"], inputs["W1"], inputs["b1"], inputs["W2"],
        inputs["b2"], inputs["W3"], inputs["b3"], l0)
    x = np.array(inputs["x"], dtype=np.float32)         # copy: mutated below
    # constant-1 bias input on each group's masked lag channel
    x.reshape(B, G * L)[:, np.arange(G) * L + l0] = 1.0
    in_maps = []
    for i in range(NCORES):
        in_maps.append({
            "x": _prep_x_shard(x[i * BS:(i + 1) * BS]),
            "w1": w1h, "w2": w2h, "w3": w3h,
            "b2s": b2s, "b2k": b2k, "b3": b3h,
        })
    res = None
    last_err = None
    for attempt in range(3):
        try:
            res = run_bass_kernel_spmd(
                nc, in_maps, list(range(NCORES)),
                trace=trace, trace_cores=trace_cores)
            break
        except Exception as e:  # transient NRT device wedge heals on rerun
            last_err = e
            time.sleep(2.0)
    if res is None:
        raise last_err
    out = np.empty((B, N), dtype=np.float32)
    for i in range(NCORES):
        out[i * BS:(i + 1) * BS] = res.results[i]["out"].T
    return out, res


def kernel(**inputs) -> np.ndarray:
    out, _ = _run(inputs, trace=False)
    return out

